# revision 64
# baseline (speedup 1.0000x reference)
"""Multi-head attention kernel for Trainium2 (8 NeuronCores, SPMD).

Sharding: core c handles batch b=c//2 and 4 of the 8 heads
(projection columns 128*(c%2) .. +128).  Each core computes a partial
output projection; the host sums the two partials per batch and adds bo.

Final structure (279us baseline -> ~208us):
  * Score matmuls for all 4 heads issued back-to-back with
    tile_position=(32h, 0): 4-way concurrent row-tiling in the PE
    array (~4x scores throughput vs sequential).
  * AV matmuls col-tiled 2-way: 33-col stationaries (v | ones column
    for the softmax denominator) at tile_position (0,0)/(0,64), two
    heads per PSUM bank.
  * Head-granular streaming: score head-slices ([128 k, 512 q]) flow
    through a ring of two [128,3,512] PSUM tiles; exp runs on ScalarE
    at FD=1536 per ring tile (~128us, the critical engine); mask-muls
    on VectorE in fp16 2x mode; AV matmuls are emitted per-head as
    soon as their weights are ready.
  * At each q-block boundary the new block's AVs are held back while
    the previous block's normalize is staged across several flushes
    (den copies, reciprocal+muls, then AV release, then the output
    projection per 512-q chunk) so the in-order PE queue never blocks
    ScalarE for long.
  * k/q/v projections are emitted just-in-time inside the j=0 stream;
    input DMAs are posted in criticality order so the first scores
    start ~15us in.
  * Output projection uses packed Wo stationaries (contraction 128);
    fp16 output, transposed [D, S]; host sums the two core-partials,
    applies the 2^-10 den compensation, and adds bo.
"""

import numpy as np
import ml_dtypes

import concourse.bass as bass
import concourse.tile as tile
from concourse import bacc, mybir
from concourse.bass_utils import run_bass_kernel_spmd
from concourse._compat import with_exitstack
from contextlib import ExitStack

B, D = 4, 256
H = 8
PROJ = 256
NCORES = 8
HPC = H // 2              # heads per core = 4
PC = HPC * 32             # projection cols per core = 128
QB = 512                  # q block
KBK = 128                 # k block

F32 = mybir.dt.float32
F16 = mybir.dt.float16
Identity = mybir.ActivationFunctionType.Identity
Exp = mybir.ActivationFunctionType.Exp
ts = bass.ts

A_SCALE = 1477.0          # mask stores {0, a}; the scale cancels in softmax


@with_exitstack
def _emit(ctx: ExitStack, tc: tile.TileContext, t: dict, S: int):
    nc = tc.nc
    NQB = S // QB             # 4
    NKB = S // KBK            # 16

    wt = ctx.enter_context(tc.tile_pool(name="wt", bufs=1))
    sb = ctx.enter_context(tc.tile_pool(name="sb", bufs=1))
    wexp = ctx.enter_context(tc.tile_pool(name="wexp", bufs=3))
    wmp = ctx.enter_context(tc.tile_pool(name="wmp", bufs=10))
    nrm = ctx.enter_context(tc.tile_pool(name="nrm", bufs=2))
    obp = ctx.enter_context(tc.tile_pool(name="obp", bufs=2))
    ring = ctx.enter_context(tc.tile_pool(name="ring", bufs=2, space="PSUM"))
    avps = ctx.enter_context(tc.tile_pool(name="avps", bufs=1, space="PSUM"))

    # ---- persistent activations ----
    qT = sb.tile([128, S], F16)
    kT = sb.tile([128, S], F16)
    vaug = sb.tile([128, HPC, NKB, 33], F16)
    oTpA = sb.tile([128, S], F16)        # rows 0-31: h0, 64-95: h1
    oTpB = sb.tile([128, S], F16)        # rows 0-31: h2, 64-95: h3
    m_sb = sb.tile([128, 2, NKB, QB], F16)
    den2 = sb.tile([128, 2, QB], F16)    # den rows at partitions 32 and 96

    # ---- constants ----
    wq_s = wt.tile([128, 2, PC], F16)
    wk_s = wt.tile([128, 2, PC], F16)
    wv_s = wt.tile([128, 2, PC], F16)
    bq_s = wt.tile([128, 1], F32)
    bk_s = wt.tile([128, 1], F32)
    bv_bc = wt.tile([128, PC], F32)
    bc2 = wt.tile([128, 128], F16)
    zst = wt.tile([128, 32], F16)        # zero stationary (av junk-row clear)
    woA = wt.tile([128, 2, 128], F16)
    woB = wt.tile([128, 2, 128], F16)
    warm = wt.tile([128, 1], F32)

    # warm up the exp table set ASAP (overlaps input DMAs)
    nc.gpsimd.memset(warm[:], 0.0)
    nc.scalar.activation(out=warm[:], in_=warm[:], func=Exp)

    # weights first (small, coalesced posts)
    nc.sync.dma_start(out=wk_s[:], in_=t["wk"][:, :].rearrange("(c p) n -> p c n", p=128))
    nc.sync.dma_start(out=wq_s[:], in_=t["wq"][:, :].rearrange("(c p) n -> p c n", p=128))
    nc.gpsimd.memset(vaug[:, :, :, 32:33], 1.0)
    nc.gpsimd.memset(oTpA[32:64, :], 0.0)
    nc.gpsimd.memset(oTpA[96:128, :], 0.0)
    nc.gpsimd.memset(oTpB[32:64, :], 0.0)
    nc.gpsimd.memset(oTpB[96:128, :], 0.0)
    nc.gpsimd.memset(den2[:, :, :], 0.0)
    nc.gpsimd.memset(zst[:, :], 0.0)

    xin = ctx.enter_context(tc.tile_pool(name="xin", bufs=1))
    xq_s = xin.tile([128, 2, S], F16)
    xk_s = xin.tile([128, 2, S], F16)
    xv_s = xin.tile([128, 2, S], F16)

    # xk sliced per j; kproj(j) right after its slices
    def proj_j(dst, xs, ws, bs, j):
        p = ring.tile([128, 3, QB], F32, tag="sc", name=f"proj{j}")
        for c in range(2):
            nc.tensor.matmul(
                p[:, 0, :], lhsT=ws[:, c, :], rhs=xs[:, c, ts(j, QB)],
                start=(c == 0), stop=(c == 1),
            )
        nc.vector.tensor_scalar_add(dst[:, ts(j, QB)], p[:, 0, :], bs[:, 0:1])

    def vproj_grp(grp, half=None):
        if half is None:
            rng = range(4 * grp, 4 * grp + 4)
        elif isinstance(half, tuple):
            rng = range(half[0], half[1])
        else:
            rng = range(4 * grp + 2 * half, 4 * grp + 2 * half + 2)
        for sbk in rng:
            p = ring.tile([128, 3, QB], F32, tag="sc", name=f"vp{sbk}")
            for c in range(2):
                nc.tensor.matmul(
                    p[:, 0, 0:PC], lhsT=xv_s[:, c, ts(sbk, 128)],
                    rhs=wv_s[:, c, :],
                    start=(c == 0), stop=(c == 1),
                )
            nc.vector.tensor_add(
                vaug[:, :, sbk, 0:32],
                p[:, 0, 0:PC].rearrange("p (h d) -> p h d", h=HPC),
                bv_bc[:, :].rearrange("p (h d) -> p h d", h=HPC),
            )

    # priority DMA order: critical prefix first, split posts for parallelism
    for c in range(2):
        nc.sync.dma_start(out=xk_s[:, c, ts(0, QB)],
                          in_=t["xk"][ts(c, 128), ts(0, QB)])
    for c in range(2):
        nc.sync.dma_start(out=xq_s[:, c, ts(0, QB)],
                          in_=t["xq"][ts(c, 128), ts(0, QB)])
    nc.sync.dma_start(out=wv_s[:], in_=t["wv"][:, :].rearrange("(c p) n -> p c n", p=128))
    for c in range(2):
        nc.sync.dma_start(out=xv_s[:, c, 0:QB],
                          in_=t["xv"][ts(c, 128), 0:QB])
    nc.sync.dma_start(out=xv_s[:, :, QB:1024],
                      in_=t["xv"][:, QB:1024].rearrange("(c p) n -> p c n", p=128))
    nc.sync.dma_start(out=bk_s[:], in_=t["bk"][:, :])
    nc.sync.dma_start(out=bq_s[:], in_=t["bq"][:, :])
    nc.sync.dma_start(
        out=m_sb[:, 0, ts(0, 4), :],
        in_=t["m01"][ts(0, 512), ts(0, QB)].rearrange("(kb p) q -> p kb q", p=128),
    )
    nc.sync.dma_start(out=bv_bc[:], in_=t["bv"].to_broadcast([128, PC]))
    nc.sync.dma_start(out=xk_s[:, :, QB:S],
                      in_=t["xk"][:, QB:S].rearrange("(c p) n -> p c n", p=128))
    nc.sync.dma_start(out=xv_s[:, :, 1024:S],
                      in_=t["xv"][:, 1024:S].rearrange("(c p) n -> p c n", p=128))
    for mc in range(1, 4):
        nc.sync.dma_start(
            out=m_sb[:, 0, ts(mc, 4), :],
            in_=t["m01"][ts(mc, 512), ts(0, QB)]
                .rearrange("(kb p) q -> p kb q", p=128),
        )
    nc.sync.dma_start(out=xq_s[:, :, QB:S],
                      in_=t["xq"][:, QB:S].rearrange("(c p) n -> p c n", p=128))
    nc.sync.dma_start(out=bc2[:], in_=t["bc2"][:, :])
    nc.sync.dma_start(out=woA[:], in_=t["woA"][:, :].rearrange("p (o n) -> p o n", o=2))
    nc.sync.dma_start(out=woB[:], in_=t["woB"][:, :].rearrange("p (o n) -> p o n", o=2))
    proj_j(kT, xk_s, wk_s, bk_s, 0)
    proj_j(qT, xq_s, wq_s, bq_s, 0)
    vproj_grp(0)

    # ---------------- attention: head-granular stream ----------------
    # head-slice g = (j*NKB + kb)*4 + h ; ring tile holds 3 head-slices.
    TILE_P = 3

    state = {
        "av": None,
        "tile": None,          # current ring tile being filled
        "w": None,
        "wm": None,
        "fill": 0,             # pairs in current tile
        "tbase": 0,            # g of slice 0
        "ready": [],           # (g, wm_tile, slice) ready for AV
        "norm_done": -1,       # last j normalized
        "op_done": -1,         # last out-proj chunk emitted
        "held": [],
        "holding": False,
        "op_pend": -1,
        "op_dc": 0,
        "nstage": 0,
        "m_done": 0,           # last mask block prefetched
        "kv_done": 0,          # last k projection block emitted
        "next_v": 4,           # next v projection sbk to emit
        "q_done": 0,           # last q projection block emitted
    }

    def flush_tile():
        """emit exp + masked muls + AV for the current ring tile."""
        n = state["fill"]
        if n == 0:
            return
        sc, tb = state["tile"], state["tbase"]
        w = wexp.tile([128, TILE_P, QB], F16, tag="w", name=f"w{tb}")
        nc.scalar.activation(out=w[0:128, 0:n, :], in_=sc[0:128, 0:n, :],
                             func=Exp)
        # masked muls grouped by (j, kb) within the tile
        s = 0
        while s < n:
            g = tb + s
            kb = (g // 4) % NKB
            j = g // (4 * NKB)
            e = s + 1
            while e < n and ((tb + e) // 4) % NKB == kb \
                    and (tb + e) // (4 * NKB) == j:
                e += 1
            wm = wmp.tile([128, TILE_P, QB], F16, tag="wm", name=f"wm{tb}_{s}")
            nc.vector.tensor_mul(
                wm[:, s:e, :],
                w[:, s:e, :],
                m_sb[:, j % 2, kb, :]
                    .rearrange("p (o nn) -> p o nn", o=1)
                    .to_broadcast([128, e - s, QB]),
            )
            for ss in range(s, e):
                state["ready"].append((tb + ss, wm, ss))
            s = e
        state["tile"] = None
        state["fill"] = 0
        drain_ready()

    def norm_copies(j):
        av = state["av"]
        nc.vector.tensor_scalar_mul(den2[:, :, :], av[:, :, :], 2.0 ** -10)

    def norm_muls(j):
        av = state["av"]
        pbc = ring.tile([128, 3, QB], F32, tag="sc", name=f"pbc{j}")
        for bank in range(2):
            nc.tensor.matmul(pbc[:, bank, :], lhsT=bc2[:, :],
                             rhs=den2[:, bank, :], start=True, stop=True)
        rec = nrm.tile([128, 2, QB], F32, tag="rec")
        nc.vector.reciprocal_approx_fast(rec[:], pbc[:, 0:2, :])
        for bank, oTp in ((0, oTpA), (1, oTpB)):
            nc.vector.tensor_mul(
                oTp[:, ts(j, QB)],
                av[:, bank, :],
                rec[:, bank, :],
            )
        state["norm_done"] = j

    def emit_outproj(j, dc):
        state["op_done"] = j
        p = ring.tile([128, 3, QB], F32, tag="sc", name=f"op{j}_{dc}")
        nc.tensor.matmul(p[:, 0, :], lhsT=woA[:, dc, :],
                         rhs=oTpA[:, ts(j, QB)],
                         start=True, stop=False)
        nc.tensor.matmul(p[:, 0, :], lhsT=woB[:, dc, :],
                         rhs=oTpB[:, ts(j, QB)],
                         start=False, stop=True)
        ob = obp.tile([128, QB], F16, tag="outbuf")
        nc.vector.tensor_copy(out=ob[:], in_=p[:, 0, :])
        nc.sync.dma_start(
            out=t["out"][ts(dc, 128), ts(j, QB)],
            in_=ob[:],
        )

    def emit_av(g, wmt, sl):
        j = g // (4 * NKB)
        kb = (g // 4) % NKB
        h = g % 4
        if kb == 0 and h == 0:
            state["av"] = avps.tile([128, 2, QB], F32, tag="av",
                                    name=f"av{j}")
            if j == 0:
                for b2 in range(2):
                    nc.tensor.matmul(state["av"][32:64, b2, :], lhsT=zst[:, :],
                                     rhs=qT[:, 0:QB], start=True, stop=True,
                                     tile_position=(0, 32))
                    nc.tensor.matmul(state["av"][96:128, b2, :], lhsT=zst[:, :],
                                     rhs=qT[:, 0:QB], start=True, stop=True,
                                     tile_position=(0, 96))
        av = state["av"]
        bank, pos = h // 2, h % 2
        nc.tensor.matmul(
            av[64 * pos:64 * pos + 33, bank, :],
            lhsT=vaug[:, h, kb, :],
            rhs=wmt[:, sl, :],
            start=(kb == 0),
            stop=(kb == NKB - 1),
            tile_position=(0, 64 * pos),
        )

    def drain_ready():
        for g, wmt, sl in state["ready"]:
            j = g // (4 * NKB)
            kb = (g // 4) % NKB
            if kb == 0 and j > 0 and state["norm_done"] < j - 1:
                state["holding"] = True
            if state["holding"]:
                state["held"].append((g, wmt, sl))
            else:
                emit_av(g, wmt, sl)
        state["ready"] = []

    def release_held(limit=None):
        n = len(state["held"]) if limit is None else min(limit, len(state["held"]))
        for e in state["held"][:n]:
            emit_av(*e)
        state["held"] = state["held"][n:]
        if not state["held"]:
            state["holding"] = False

    def prefetch_mask(j1):
        nc.sync.dma_start(
            out=m_sb[:, j1 % 2, :, :],
            in_=t["m01"][:, ts(j1, QB)].rearrange("(kb p) q -> p kb q", p=128),
        )

    for j in range(NQB):
        jb = j % 2
        for kb in range(NKB):
            for h in range(HPC):
                if state["tile"] is None:
                    g = (j * NKB + kb) * 4 + h
                    state["tile"] = ring.tile([128, TILE_P, QB], F32,
                                              tag="sc", name=f"sc{g}")
                    state["tbase"] = g
                sl = state["fill"]
                nc.tensor.matmul(
                    state["tile"][:, sl, :],
                    lhsT=kT[32 * h:32 * h + 32, ts(kb, KBK)],
                    rhs=qT[32 * h:32 * h + 32, ts(j, QB)],
                    start=True, stop=True,
                    tile_position=(32 * h, 0),
                )
                state["fill"] += 1
                if state["fill"] == TILE_P:
                    flush_tile()
                    if state["holding"] and j > 0:
                        if kb >= 2 and state["nstage"] < 1:
                            state["nstage"] = 1
                            norm_copies(j - 1)
                        elif kb >= 3 and state["nstage"] < 2:
                            state["nstage"] = 2
                            norm_muls(j - 1)
                        elif kb >= 4 and state["nstage"] < 3:
                            release_held()
                            state["nstage"] = 3
                            state["op_pend"] = j - 1
                    if state["op_pend"] >= 0 and kb >= 7 and state["op_dc"] < 1:
                        state["op_dc"] = 1
                        emit_outproj(state["op_pend"], 0)
                    elif state["op_pend"] >= 0 and kb >= 9:
                        emit_outproj(state["op_pend"], 1)
                        state["op_pend"] = -1
                        state["op_dc"] = 0
                        state["nstage"] = 0
                    if kb >= 1 and j + 1 < NQB and state["m_done"] <= j:
                        state["m_done"] = j + 1
                        prefetch_mask(j + 1)
                    if j == 0:
                        if kb >= 2 and state["kv_done"] < 1:
                            state["kv_done"] = 1
                            proj_j(kT, xk_s, wk_s, bk_s, 1)
                        elif kb >= 6 and state["kv_done"] < 2:
                            state["kv_done"] = 2
                            proj_j(kT, xk_s, wk_s, bk_s, 2)
                        elif kb >= 10 and state["kv_done"] < 3:
                            state["kv_done"] = 3
                            proj_j(kT, xk_s, wk_s, bk_s, 3)
                        elif state["next_v"] < 16 \
                                and kb >= state["next_v"] - 3:
                            vproj_grp(0, (state["next_v"],
                                          state["next_v"] + 1))
                            state["next_v"] += 1
                    if kb >= 13 and j + 1 < NQB and state["q_done"] <= j:
                        state["q_done"] = j + 1
                        proj_j(qT, xq_s, wq_s, bq_s, j + 1)
    flush_tile()
    norm_copies(NQB - 1)
    norm_muls(NQB - 1)
    if state["op_pend"] >= 0:
        emit_outproj(state["op_pend"], 0)
        emit_outproj(state["op_pend"], 1)
    emit_outproj(NQB - 1, 0)
    emit_outproj(NQB - 1, 1)


def build(S: int = 2048):
    nc = bacc.Bacc("TRN2", target_bir_lowering=False, debug=False,
                   num_devices=NCORES)
    t = {}
    t["xq"] = nc.dram_tensor("xq", [D, S], F16, kind="ExternalInput").ap()
    t["xk"] = nc.dram_tensor("xk", [D, S], F16, kind="ExternalInput").ap()
    t["xv"] = nc.dram_tensor("xv", [D, S], F16, kind="ExternalInput").ap()
    t["wq"] = nc.dram_tensor("wq", [D, PC], F16, kind="ExternalInput").ap()
    t["wk"] = nc.dram_tensor("wk", [D, PC], F16, kind="ExternalInput").ap()
    t["wv"] = nc.dram_tensor("wv", [D, PC], F16, kind="ExternalInput").ap()
    t["woA"] = nc.dram_tensor("woA", [128, D], F16, kind="ExternalInput").ap()
    t["woB"] = nc.dram_tensor("woB", [128, D], F16, kind="ExternalInput").ap()
    t["bc2"] = nc.dram_tensor("bc2", [128, 128], F16, kind="ExternalInput").ap()
    t["bq"] = nc.dram_tensor("bq", [PC, 1], F32, kind="ExternalInput").ap()
    t["bk"] = nc.dram_tensor("bk", [PC, 1], F32, kind="ExternalInput").ap()
    t["bv"] = nc.dram_tensor("bv", [1, PC], F32, kind="ExternalInput").ap()
    t["m01"] = nc.dram_tensor("m01", [S, S], F16, kind="ExternalInput").ap()
    t["out"] = nc.dram_tensor("out", [D, S], F16, kind="ExternalOutput").ap()

    with tile.TileContext(nc) as tc:
        _emit(tc, t, S)
    nc.compile()
    return nc


_NC_CACHE = {}


def _get_nc(S):
    if S not in _NC_CACHE:
        _NC_CACHE[S] = build(S)
    return _NC_CACHE[S]


def make_in_maps(queries, keys, values, mask, Wq, bq, Wk, bk, Wv, bv, Wo, bo):
    queries = np.asarray(queries, np.float32)
    keys = np.asarray(keys, np.float32)
    values = np.asarray(values, np.float32)
    mask = np.asarray(mask)
    Wq, Wk, Wv, Wo = (np.asarray(a, np.float32) for a in (Wq, Wk, Wv, Wo))
    bq, bk, bv, bo = (np.asarray(a, np.float32) for a in (bq, bk, bv, bo))
    S = queries.shape[1]
    sc = np.float32(1.0) / np.sqrt(np.float32(PROJ))
    f16 = np.float16

    bc2 = np.zeros((128, 128), f16)
    bc2[32, 0:64] = 1.0
    bc2[96, 64:128] = 1.0

    in_maps = []
    for c in range(NCORES):
        b = c // 2
        p0 = PC * (c % 2)
        m01 = (mask[b, 0].T.astype(np.float32) * A_SCALE).astype(f16)
        woA = np.zeros((128, D), np.float32)
        woB = np.zeros((128, D), np.float32)
        woA[0:32] = Wo[p0 + 0 * 32: p0 + 1 * 32, :]
        woA[64:96] = Wo[p0 + 1 * 32: p0 + 2 * 32, :]
        woB[0:32] = Wo[p0 + 2 * 32: p0 + 3 * 32, :]
        woB[64:96] = Wo[p0 + 3 * 32: p0 + 4 * 32, :]
        im = {
            "xq": np.ascontiguousarray(queries[b].T).astype(f16),
            "xk": np.ascontiguousarray(keys[b].T).astype(f16),
            "xv": np.ascontiguousarray(values[b].T).astype(f16),
            "wq": (Wq[:, p0:p0 + PC] * sc).astype(f16),
            "wk": Wk[:, p0:p0 + PC].astype(f16),
            "wv": Wv[:, p0:p0 + PC].astype(f16),
            "bq": np.ascontiguousarray((bq[p0:p0 + PC] * sc).reshape(PC, 1)),
            "bk": np.ascontiguousarray(bk[p0:p0 + PC].reshape(PC, 1)),
            "bv": np.ascontiguousarray(bv[p0:p0 + PC].reshape(1, PC)),
            "m01": m01,
            "woA": woA.astype(f16),
            "woB": woB.astype(f16),
            "bc2": bc2,
        }
        in_maps.append(im)
    return in_maps


def run(inputs, trace=False):
    S = np.asarray(inputs["queries"]).shape[1]
    nc = _get_nc(S)
    in_maps = make_in_maps(**inputs)
    res = run_bass_kernel_spmd(nc, in_maps, core_ids=list(range(NCORES)),
                               trace=trace)
    parts = [np.asarray(r["out"], np.float32) for r in res.results]
    bo = np.asarray(inputs["bo"], np.float32)
    out = np.zeros((B, S, D), np.float32)
    for b in range(B):
        out[b] = (parts[2 * b] + parts[2 * b + 1]).T * np.float32(2.0 ** -10) \
            + bo[None, :]
    return out, res


def kernel(**inputs) -> np.ndarray:
    out, _ = run(inputs, trace=False)
    return out
